# revision 2
# baseline (speedup 1.0000x reference)
"""Trainium2 Bass kernel for nn_BotUpSaliency (B=2, H=W=512, K=12, 16 steps).

Math
----
The reference integrates, for 16 Euler steps (EPS=0.01):

    y'  = y + EPS*(-y + gx + conv(gx,W) + 1)
    x'  = x + EPS*(J0*gx + conv(gx,J) + inputs + i_norm - x - gy - gy@psi)
    gx  = clip(x - 1, 0, 1),  gy piecewise-linear,  out = mean_t gx_t, max over K

with x0 = 0.01, y0 = 1.  While gx == 0 (everywhere), the system collapses
exactly:
  * y stays exactly 1.0  (y + 0.01*(-1 + 0 + 0 + 1) == y), so gy == 0.21.
  * i_norm == 0.85 (conv of the all-zero s), conv(gx,*) == 0.
  * x_t = a_t * inputs + b_t elementwise, with scalar recurrences
        a_{t+1} = (1-EPS) a_t + EPS,           a_0 = 0
        b_{t+1} = (1-EPS) b_t + EPS*(0.85 - gy - colsum(psi)*gy),  b_0 = 0.01
  * gx_t = clip(a_t*inputs + b_t - 1, 0, 1) stays identically 0 as long as
        max_t (a_t * inputs.max() + b_t) < 1
    which requires inputs.max() >= ~6.66; the model's input domain is [0,1).

Hence out = (1/16) * sum_t clip(a_t*inputs + b_t - 1, 0, 1), evaluated at
m = max_k inputs (each term is nondecreasing in the input, so the channel max
commutes with the whole expression).  The clip knots (1-b_t)/a_t decrease
with t, so for m below the t=16 knot ((1-b_15)/a_15 ~= 6.66) the sum equals
its t=16 term alone, which stays far below the upper clip:

    out = (a_16/16) * max(m - knot, 0),   knot = (1 - b_16)/a_16

The device computes the memory-bound part exactly: m = per-pixel channel max
of the bf16-staged input slab (reads all input bytes), then z = max(m - knot,
0); the exact scale a_16/16 is applied on the host in f32 (z is identically 0
on the guard-certified domain, so no precision is lost).  A host-side guard
verifies the collapse precondition (with wide margin, covering the bf16
staging round-up) and otherwise falls back to the full reference math on CPU.

Device program (raw bass, no TileContext — measured ~18us vs 21.8us for the
Tile-scheduled version):
  * input staged channel-major: 12 planes of [128 rows, 512 cols] bf16
  * 6 chunk DMAs (2,2,2,2,3,1 planes) issued back-to-back, alternating the
    two HWDGE rings (scalar first: its sequencer exits the NEFF preamble
    ~0.7us before sync's)
  * DVE folds each chunk to a 512-wide running max as its completion
    semaphore fires; the final 1-plane chunk is folded and relu'd in column
    halves so the first output DMA trigger overlaps the second half's work
  * no final completion wait: the ~7us runtime postamble drains the output
    DMA (validated over repeated runs)

Robustness: the PJRT/axon input copy-in RACES with NEFF execution — on the
first-ever execution the kernel can read uninitialized HBM (garbage above the
knot => wrong nonzero output; observed on the staged baseline too).  kernel()
therefore always runs a warmup execution with the same inputs before the run
whose outputs are returned: the warmup forces the staged bytes into HBM, and
any tear on the second run rewrites identical bytes, making it exact.

Sharding: pure data parallelism, 8 cores x 128 rows of the flattened
(2*512, 512, 12) input.
"""

from contextlib import ExitStack

import numpy as np

K = 12
STEPS = 16
EPS = 0.01
TX = 1.0
G1 = 0.21
J0 = 0.8
B, H, WD = 2, 512, 512
N_CORES = 8
ROWS = B * H                  # 1024 flattened rows
RPC = ROWS // N_CORES         # 128 rows per core == SBUF partitions
ROWW = WD * K                 # 6144 elements per row, channel-major
CHUNK_PLANES = (2, 2, 2, 2, 3, 1)
assert sum(CHUNK_PLANES) == K

_CACHE = {}


def _coeffs(colsum):
    """Scalar affine recurrence coefficients while gx == 0 (float64)."""
    gy = G1 * 1.0             # y stays exactly 1.0
    drive = 0.85 - gy - colsum * gy
    a, b = 0.0, 0.01
    A, Bc = [], []
    for _ in range(STEPS):
        a = (1.0 - EPS) * a + EPS
        b = (1.0 - EPS) * b + EPS * drive
        A.append(a)
        Bc.append(b)
    return np.array(A), np.array(Bc)


def _build_program(knot):
    import concourse.bacc as bacc
    import concourse.bass as bass
    import concourse.mybir as mybir

    bf16 = mybir.dt.bfloat16
    MAX = mybir.AluOpType.max
    ADD = mybir.AluOpType.add

    def sb_ap(t, ncols, offset=0, cols=None):
        cols = ncols if cols is None else cols
        return bass.AP(t, offset, [[ncols, RPC], [1, cols]])

    nc = bacc.Bacc("TRN2", target_bir_lowering=False, debug=False)
    x = nc.dram_tensor("x", [RPC, ROWW], bf16, kind="ExternalInput")
    out = nc.dram_tensor("out", [RPC, WD], bf16, kind="ExternalOutput")
    engs = [nc.scalar, nc.sync]
    with ExitStack() as ctx:
        xt = ctx.enter_context(nc.sbuf_tensor([RPC, ROWW], bf16))
        tmp = ctx.enter_context(nc.sbuf_tensor([RPC, WD], bf16))
        ft = ctx.enter_context(nc.sbuf_tensor([RPC, WD], bf16))
        r0 = ctx.enter_context(nc.sbuf_tensor([RPC, WD], bf16))
        r1 = ctx.enter_context(nc.sbuf_tensor([RPC, WD], bf16))
        zt = ctx.enter_context(nc.sbuf_tensor([RPC, WD], bf16))
        sems = [nc.alloc_semaphore("s_e0"), nc.alloc_semaphore("s_e1")]
        s_d = nc.alloc_semaphore("s_d")
        s_o = nc.alloc_semaphore("s_o")

        offs = [0]
        for npl in CHUNK_PLANES:
            offs.append(offs[-1] + npl * WD)
        cnt = [0, 0]
        chunk_sem = []
        for c, npl in enumerate(CHUNK_PLANES):
            e = c % 2
            engs[e].dma_start(
                out=sb_ap(xt, ROWW, offs[c], npl * WD),
                in_=x[:, offs[c]:offs[c + 1]]).then_inc(sems[e], 16)
            cnt[e] += 1
            chunk_sem.append((sems[e], 16 * cnt[e]))

        rcur = None
        for c, npl in enumerate(CHUNK_PLANES[:-1]):
            sem, val = chunk_sem[c]
            nc.vector.wait_ge(sem, val)
            base = offs[c]
            dstf = sb_ap(r0, WD) if rcur is None else sb_ap(ft, WD)
            if npl == 2:
                nc.vector.tensor_tensor(
                    out=dstf, in0=sb_ap(xt, ROWW, base, WD),
                    in1=sb_ap(xt, ROWW, base + WD, WD), op=MAX)
            elif npl == 3:
                t = sb_ap(tmp, WD)
                nc.vector.tensor_tensor(
                    out=t, in0=sb_ap(xt, ROWW, base, WD),
                    in1=sb_ap(xt, ROWW, base + WD, WD), op=MAX)
                nc.vector.tensor_tensor(
                    out=dstf, in0=t,
                    in1=sb_ap(xt, ROWW, base + 2 * WD, WD), op=MAX)
            else:
                raise NotImplementedError(npl)
            if rcur is None:
                rcur = dstf
            else:
                rnext = sb_ap(r1 if rcur.tensor is r0 else r0, WD)
                nc.vector.tensor_tensor(out=rnext, in0=rcur, in1=dstf, op=MAX)
                rcur = rnext

        # final 1-plane chunk: run-max + relu in halves; out DMA per half
        half = WD // 2
        sem, val = chunk_sem[-1]
        nc.vector.wait_ge(sem, val)
        base = offs[len(CHUNK_PLANES) - 1]
        m0 = bass.AP(tmp, 0, [[WD, RPC], [1, half]])
        m1 = bass.AP(tmp, half, [[WD, RPC], [1, half]])
        z0 = bass.AP(zt, 0, [[WD, RPC], [1, half]])
        z1 = bass.AP(zt, half, [[WD, RPC], [1, half]])

        def rh(off, w):
            return bass.AP(rcur.tensor, rcur.offset + off, [[WD, RPC], [1, w]])

        def lastp(off, w):
            return bass.AP(xt, base + off, [[ROWW, RPC], [1, w]])

        nc.vector.tensor_tensor(out=m0, in0=rh(0, half), in1=lastp(0, half), op=MAX)
        nc.vector.tensor_scalar(out=z0, in0=m0, scalar1=float(-knot), scalar2=0.0,
                                op0=ADD, op1=MAX).then_inc(s_d, 1)
        nc.vector.tensor_tensor(out=m1, in0=rh(half, half), in1=lastp(half, half),
                                op=MAX)
        nc.vector.tensor_scalar(out=z1, in0=m1, scalar1=float(-knot), scalar2=0.0,
                                op0=ADD, op1=MAX).then_inc(s_d, 2)
        engs[0].wait_ge(s_d, 1)
        engs[0].dma_start(out=out[:, :half], in_=z0).then_inc(s_o, 16)
        engs[1].wait_ge(s_d, 3)
        engs[1].dma_start(out=out[:, half:], in_=z1).then_inc(s_o, 16)
    nc.compile()
    return nc


def _get_program(knot):
    key = round(float(knot), 12)
    if key not in _CACHE:
        _CACHE[key] = _build_program(knot)
    return _CACHE[key]


def _run_on_device(inputs_np, A, Bc, trace=False):
    from concourse.bass_utils import run_bass_kernel_spmd
    import ml_dtypes

    knot = (1.0 - Bc[STEPS - 1]) / A[STEPS - 1]
    alpha = A[STEPS - 1] / STEPS
    nc = _get_program(knot)
    flat = np.ascontiguousarray(
        inputs_np.reshape(ROWS, WD, K).transpose(0, 2, 1)
    ).astype(ml_dtypes.bfloat16).reshape(ROWS, ROWW)
    in_maps = [
        {"x": np.ascontiguousarray(flat[i * RPC:(i + 1) * RPC])}
        for i in range(N_CORES)
    ]
    # Warmup execution: forces the staged input bytes into device HBM so the
    # measured run cannot read uninitialized memory if its own copy-in races
    # with NEFF execution (observed; see module docstring).
    run_bass_kernel_spmd(nc, in_maps, list(range(N_CORES)))
    res = run_bass_kernel_spmd(nc, in_maps, list(range(N_CORES)), trace=trace)
    z = np.concatenate(
        [res.results[i]["out"].astype(np.float32) for i in range(N_CORES)], axis=0)
    out = (z * np.float32(alpha)).reshape(B, H, WD).astype(np.float32)
    return out, res


def _reference_fallback(inputs, Wk, Jk, psi):
    """Full reference math in jax on CPU (only for out-of-domain inputs)."""
    import jax
    import jax.numpy as jnp

    cpu = jax.devices("cpu")[0]
    with jax.default_device(cpu):
        inputs = jnp.asarray(np.asarray(inputs), jnp.float32)
        Wk = jnp.asarray(np.asarray(Wk), jnp.float32)
        Jk = jnp.asarray(np.asarray(Jk), jnp.float32)
        psi = jnp.asarray(np.asarray(psi), jnp.float32)
        PAD = 7

        def _conv(xx, kk, padding):
            return jax.lax.conv_general_dilated(
                xx, kk, (1, 1), padding,
                dimension_numbers=("NHWC", "HWIO", "NHWC"))

        def _gx(xx):
            return jnp.clip(xx - TX, 0.0, 1.0)

        def _gy(yy):
            yc = jnp.maximum(yy, 0.0)
            return jnp.where(yc <= 1.2, G1 * yc, G1 * 1.2 + 2.5 * (yc - 1.2))

        psi_mat = psi[0, 0]
        box = jnp.ones((5, 5, 1, 1), inputs.dtype)
        x = jnp.full_like(inputs, 0.01)
        y = jnp.ones_like(inputs)
        gx = _gx(x)
        gy = _gy(y)
        out = jnp.zeros_like(inputs)
        for _ in range(STEPS):
            s = jnp.sum(gx, axis=3, keepdims=True)
            i_norm = 0.85 - 2.0 * (_conv(s, box, "SAME") / 25.0) ** 2
            gx_p = jnp.pad(gx, ((0, 0), (PAD, PAD), (PAD, PAD), (0, 0)),
                           mode="symmetric")
            inhib = _conv(gx_p, Wk, "VALID")
            excit = _conv(gx_p, Jk, "VALID")
            inhibs_psi = jnp.einsum("bhwi,io->bhwo", gy, psi_mat)
            y_new = y + EPS * (-y + gx + inhib + 1.0)
            x_inhib = x + gy + inhibs_psi
            x_excit = J0 * gx + excit + inputs + i_norm
            x_new = x + EPS * (x_excit - x_inhib)
            gx = _gx(x_new)
            gy = _gy(y_new)
            x, y = x_new, y_new
            out = out + gx
        out = out / STEPS
        return np.asarray(jnp.max(out, axis=3))


def kernel(inputs, W=None, J=None, psi=None, **_ignored):
    inputs_np = np.asarray(inputs, dtype=np.float32)
    assert inputs_np.shape == (B, H, WD, K), inputs_np.shape

    # Guard: the gx==0 collapse must hold for these inputs/psi.
    ok = True
    colsum = 3.0
    if psi is not None:
        cs = np.asarray(psi, dtype=np.float64)[0, 0].sum(axis=0)
        if np.max(np.abs(cs - cs[0])) < 1e-9:
            colsum = float(cs[0])
        else:
            ok = False
    if ok:
        A, Bc = _coeffs(colsum)
        # 1.004 factor covers bf16 round-up of the staged inputs (<= 2^-8 rel)
        mx = float(inputs_np.max()) * 1.004
        if np.max(A * mx + Bc) >= 0.98:
            ok = False
    if not ok:
        return _reference_fallback(inputs, W, J, psi).astype(np.float32)

    out, _ = _run_on_device(inputs_np, A, Bc)
    return out


if __name__ == "__main__":
    rng = np.random.default_rng(0)
    x = rng.random((B, H, WD, K), dtype=np.float32)
    o = kernel(inputs=x)
    print("kernel out:", o.shape, o.dtype, "maxabs", np.abs(o).max())


# revision 4
# speedup vs baseline: 1.1167x; 1.1167x over previous
"""Trainium2 Bass kernel for nn_BotUpSaliency (B=2, H=W=512, K=12, 16 steps).

Math
----
The reference integrates, for 16 Euler steps (EPS=0.01):

    y'  = y + EPS*(-y + gx + conv(gx,W) + 1)
    x'  = x + EPS*(J0*gx + conv(gx,J) + inputs + i_norm - x - gy - gy@psi)
    gx  = clip(x - 1, 0, 1),  gy piecewise-linear,  out = mean_t gx_t, max over K

with x0 = 0.01, y0 = 1.  While gx == 0 (everywhere), the system collapses
exactly:
  * y stays exactly 1.0  (y + 0.01*(-1 + 0 + 0 + 1) == y), so gy == 0.21.
  * i_norm == 0.85 (conv of the all-zero s), conv(gx,*) == 0.
  * x_t = a_t * inputs + b_t elementwise, with scalar recurrences
        a_{t+1} = (1-EPS) a_t + EPS,           a_0 = 0
        b_{t+1} = (1-EPS) b_t + EPS*(0.85 - gy - colsum(psi)*gy),  b_0 = 0.01
  * gx_t = clip(a_t*inputs + b_t - 1, 0, 1) stays identically 0 as long as
        max_t (a_t * inputs.max() + b_t) < 1
    which requires inputs.max() >= ~6.66; the model's input domain is [0,1).

Hence out = (1/16) * sum_t clip(a_t*inputs + b_t - 1, 0, 1), evaluated at
m = max_k inputs (each term is nondecreasing in the input, so the channel max
commutes with the whole expression).  The clip knots (1-b_t)/a_t decrease
with t, so for m below the t=16 knot ((1-b_15)/a_15 ~= 6.66) the sum equals
its t=16 term alone, which stays far below the upper clip:

    out = (a_16/16) * max(m - knot, 0),   knot = (1 - b_16)/a_16

The device computes the memory-bound part exactly: m = per-pixel channel max
of the bf16-staged input slab (reads all input bytes), then z = max(m - knot,
0); the exact scale a_16/16 is applied on the host in f32 (z is identically 0
on the guard-certified domain, so no precision is lost).  A host-side guard
verifies the collapse precondition (with wide margin, covering the bf16
staging round-up) and otherwise falls back to the full reference math on CPU.

Device program (raw bass, no TileContext — measured ~18us vs 21.8us for the
Tile-scheduled version):
  * input staged channel-major: 12 planes of [128 rows, 512 cols] bf16
  * 6 chunk DMAs (3,2,2,2,2,1 planes) issued back-to-back, alternating the
    two HWDGE rings (scalar first: its sequencer exits the NEFF preamble
    ~0.7us before sync's).  Big chunk first / tiny chunk last: DMA-completion
    semaphores take ~1.2-1.9us to become visible to the waiting engine, so
    the last-arriving chunks must need the fewest DVE ops after their wake
  * DVE folds each chunk to a 512-wide running max as its completion
    semaphore fires; the final 1-plane chunk is folded and relu'd in column
    halves so the first output DMA trigger overlaps the second half's work
  * no final completion wait: the ~7us runtime postamble drains the output
    DMA (validated over repeated runs)

Robustness: the PJRT/axon input copy-in RACES with NEFF execution — on the
first-ever execution the kernel can read uninitialized HBM (garbage above the
knot => wrong nonzero output; observed on the staged baseline too).  kernel()
therefore always runs a warmup execution with the same inputs before the run
whose outputs are returned: the warmup forces the staged bytes into HBM, and
any tear on the second run rewrites identical bytes, making it exact.

Sharding: pure data parallelism, 8 cores x 128 rows of the flattened
(2*512, 512, 12) input.
"""

from contextlib import ExitStack

import numpy as np

K = 12
STEPS = 16
EPS = 0.01
TX = 1.0
G1 = 0.21
J0 = 0.8
B, H, WD = 2, 512, 512
N_CORES = 8
ROWS = B * H                  # 1024 flattened rows
RPC = ROWS // N_CORES         # 128 rows per core == SBUF partitions
ROWW = WD * K                 # 6144 elements per row, channel-major
CHUNK_PLANES = (3, 2, 2, 2, 2, 1)
assert sum(CHUNK_PLANES) == K

_CACHE = {}


def _coeffs(colsum):
    """Scalar affine recurrence coefficients while gx == 0 (float64)."""
    gy = G1 * 1.0             # y stays exactly 1.0
    drive = 0.85 - gy - colsum * gy
    a, b = 0.0, 0.01
    A, Bc = [], []
    for _ in range(STEPS):
        a = (1.0 - EPS) * a + EPS
        b = (1.0 - EPS) * b + EPS * drive
        A.append(a)
        Bc.append(b)
    return np.array(A), np.array(Bc)


def _build_program(knot):
    import concourse.bacc as bacc
    import concourse.bass as bass
    import concourse.mybir as mybir

    bf16 = mybir.dt.bfloat16
    MAX = mybir.AluOpType.max
    ADD = mybir.AluOpType.add

    def sb_ap(t, ncols, offset=0, cols=None):
        cols = ncols if cols is None else cols
        return bass.AP(t, offset, [[ncols, RPC], [1, cols]])

    nc = bacc.Bacc("TRN2", target_bir_lowering=False, debug=False)
    x = nc.dram_tensor("x", [RPC, ROWW], bf16, kind="ExternalInput")
    out = nc.dram_tensor("out", [RPC, WD], bf16, kind="ExternalOutput")
    engs = [nc.scalar, nc.sync]
    with ExitStack() as ctx:
        xt = ctx.enter_context(nc.sbuf_tensor([RPC, ROWW], bf16))
        tmp = ctx.enter_context(nc.sbuf_tensor([RPC, WD], bf16))
        ft = ctx.enter_context(nc.sbuf_tensor([RPC, WD], bf16))
        r0 = ctx.enter_context(nc.sbuf_tensor([RPC, WD], bf16))
        r1 = ctx.enter_context(nc.sbuf_tensor([RPC, WD], bf16))
        zt = ctx.enter_context(nc.sbuf_tensor([RPC, WD], bf16))
        sems = [nc.alloc_semaphore("s_e0"), nc.alloc_semaphore("s_e1")]
        s_d = nc.alloc_semaphore("s_d")
        s_o = nc.alloc_semaphore("s_o")

        offs = [0]
        for npl in CHUNK_PLANES:
            offs.append(offs[-1] + npl * WD)
        cnt = [0, 0]
        chunk_sem = []
        for c, npl in enumerate(CHUNK_PLANES):
            e = c % 2
            engs[e].dma_start(
                out=sb_ap(xt, ROWW, offs[c], npl * WD),
                in_=x[:, offs[c]:offs[c + 1]]).then_inc(sems[e], 16)
            cnt[e] += 1
            chunk_sem.append((sems[e], 16 * cnt[e]))

        rcur = None
        for c, npl in enumerate(CHUNK_PLANES[:-1]):
            sem, val = chunk_sem[c]
            nc.vector.wait_ge(sem, val)
            base = offs[c]
            dstf = sb_ap(r0, WD) if rcur is None else sb_ap(ft, WD)
            if npl == 2:
                nc.vector.tensor_tensor(
                    out=dstf, in0=sb_ap(xt, ROWW, base, WD),
                    in1=sb_ap(xt, ROWW, base + WD, WD), op=MAX)
            elif npl == 3:
                t = sb_ap(tmp, WD)
                nc.vector.tensor_tensor(
                    out=t, in0=sb_ap(xt, ROWW, base, WD),
                    in1=sb_ap(xt, ROWW, base + WD, WD), op=MAX)
                nc.vector.tensor_tensor(
                    out=dstf, in0=t,
                    in1=sb_ap(xt, ROWW, base + 2 * WD, WD), op=MAX)
            else:
                raise NotImplementedError(npl)
            if rcur is None:
                rcur = dstf
            else:
                rnext = sb_ap(r1 if rcur.tensor is r0 else r0, WD)
                nc.vector.tensor_tensor(out=rnext, in0=rcur, in1=dstf, op=MAX)
                rcur = rnext

        # final 1-plane chunk: run-max + relu in halves; out DMA per half
        half = WD // 2
        sem, val = chunk_sem[-1]
        nc.vector.wait_ge(sem, val)
        base = offs[len(CHUNK_PLANES) - 1]
        m0 = bass.AP(tmp, 0, [[WD, RPC], [1, half]])
        m1 = bass.AP(tmp, half, [[WD, RPC], [1, half]])
        z0 = bass.AP(zt, 0, [[WD, RPC], [1, half]])
        z1 = bass.AP(zt, half, [[WD, RPC], [1, half]])

        def rh(off, w):
            return bass.AP(rcur.tensor, rcur.offset + off, [[WD, RPC], [1, w]])

        def lastp(off, w):
            return bass.AP(xt, base + off, [[ROWW, RPC], [1, w]])

        nc.vector.tensor_tensor(out=m0, in0=rh(0, half), in1=lastp(0, half), op=MAX)
        nc.vector.tensor_scalar(out=z0, in0=m0, scalar1=float(-knot), scalar2=0.0,
                                op0=ADD, op1=MAX).then_inc(s_d, 1)
        nc.vector.tensor_tensor(out=m1, in0=rh(half, half), in1=lastp(half, half),
                                op=MAX)
        nc.vector.tensor_scalar(out=z1, in0=m1, scalar1=float(-knot), scalar2=0.0,
                                op0=ADD, op1=MAX).then_inc(s_d, 2)
        engs[0].wait_ge(s_d, 1)
        engs[0].dma_start(out=out[:, :half], in_=z0).then_inc(s_o, 16)
        engs[1].wait_ge(s_d, 3)
        engs[1].dma_start(out=out[:, half:], in_=z1).then_inc(s_o, 16)
    nc.compile()
    return nc


def _get_program(knot):
    key = round(float(knot), 12)
    if key not in _CACHE:
        _CACHE[key] = _build_program(knot)
    return _CACHE[key]


def _run_on_device(inputs_np, A, Bc, trace=False):
    from concourse.bass_utils import run_bass_kernel_spmd
    import ml_dtypes

    knot = (1.0 - Bc[STEPS - 1]) / A[STEPS - 1]
    alpha = A[STEPS - 1] / STEPS
    nc = _get_program(knot)
    flat = np.ascontiguousarray(
        inputs_np.reshape(ROWS, WD, K).transpose(0, 2, 1)
    ).astype(ml_dtypes.bfloat16).reshape(ROWS, ROWW)
    in_maps = [
        {"x": np.ascontiguousarray(flat[i * RPC:(i + 1) * RPC])}
        for i in range(N_CORES)
    ]
    # Warmup execution: forces the staged input bytes into device HBM so the
    # measured run cannot read uninitialized memory if its own copy-in races
    # with NEFF execution (observed; see module docstring).
    run_bass_kernel_spmd(nc, in_maps, list(range(N_CORES)))
    res = run_bass_kernel_spmd(nc, in_maps, list(range(N_CORES)), trace=trace)
    z = np.concatenate(
        [res.results[i]["out"].astype(np.float32) for i in range(N_CORES)], axis=0)
    out = (z * np.float32(alpha)).reshape(B, H, WD).astype(np.float32)
    return out, res


def _reference_fallback(inputs, Wk, Jk, psi):
    """Full reference math in jax on CPU (only for out-of-domain inputs)."""
    import jax
    import jax.numpy as jnp

    cpu = jax.devices("cpu")[0]
    with jax.default_device(cpu):
        inputs = jnp.asarray(np.asarray(inputs), jnp.float32)
        Wk = jnp.asarray(np.asarray(Wk), jnp.float32)
        Jk = jnp.asarray(np.asarray(Jk), jnp.float32)
        psi = jnp.asarray(np.asarray(psi), jnp.float32)
        PAD = 7

        def _conv(xx, kk, padding):
            return jax.lax.conv_general_dilated(
                xx, kk, (1, 1), padding,
                dimension_numbers=("NHWC", "HWIO", "NHWC"))

        def _gx(xx):
            return jnp.clip(xx - TX, 0.0, 1.0)

        def _gy(yy):
            yc = jnp.maximum(yy, 0.0)
            return jnp.where(yc <= 1.2, G1 * yc, G1 * 1.2 + 2.5 * (yc - 1.2))

        psi_mat = psi[0, 0]
        box = jnp.ones((5, 5, 1, 1), inputs.dtype)
        x = jnp.full_like(inputs, 0.01)
        y = jnp.ones_like(inputs)
        gx = _gx(x)
        gy = _gy(y)
        out = jnp.zeros_like(inputs)
        for _ in range(STEPS):
            s = jnp.sum(gx, axis=3, keepdims=True)
            i_norm = 0.85 - 2.0 * (_conv(s, box, "SAME") / 25.0) ** 2
            gx_p = jnp.pad(gx, ((0, 0), (PAD, PAD), (PAD, PAD), (0, 0)),
                           mode="symmetric")
            inhib = _conv(gx_p, Wk, "VALID")
            excit = _conv(gx_p, Jk, "VALID")
            inhibs_psi = jnp.einsum("bhwi,io->bhwo", gy, psi_mat)
            y_new = y + EPS * (-y + gx + inhib + 1.0)
            x_inhib = x + gy + inhibs_psi
            x_excit = J0 * gx + excit + inputs + i_norm
            x_new = x + EPS * (x_excit - x_inhib)
            gx = _gx(x_new)
            gy = _gy(y_new)
            x, y = x_new, y_new
            out = out + gx
        out = out / STEPS
        return np.asarray(jnp.max(out, axis=3))


def kernel(inputs, W=None, J=None, psi=None, **_ignored):
    inputs_np = np.asarray(inputs, dtype=np.float32)
    assert inputs_np.shape == (B, H, WD, K), inputs_np.shape

    # Guard: the gx==0 collapse must hold for these inputs/psi.
    ok = True
    colsum = 3.0
    if psi is not None:
        cs = np.asarray(psi, dtype=np.float64)[0, 0].sum(axis=0)
        if np.max(np.abs(cs - cs[0])) < 1e-9:
            colsum = float(cs[0])
        else:
            ok = False
    if ok:
        A, Bc = _coeffs(colsum)
        # 1.004 factor covers bf16 round-up of the staged inputs (<= 2^-8 rel)
        mx = float(inputs_np.max()) * 1.004
        if np.max(A * mx + Bc) >= 0.98:
            ok = False
    if not ok:
        return _reference_fallback(inputs, W, J, psi).astype(np.float32)

    out, _ = _run_on_device(inputs_np, A, Bc)
    return out


if __name__ == "__main__":
    rng = np.random.default_rng(0)
    x = rng.random((B, H, WD, K), dtype=np.float32)
    o = kernel(inputs=x)
    print("kernel out:", o.shape, o.dtype, "maxabs", np.abs(o).max())
